# revision 24
# baseline (speedup 1.0000x reference)
"""Tensor-parallel MoE GroupedMLP kernel for 8 Trainium2 NeuronCores.

Problem: T=4096 tokens, H=2048 hidden, E=8 experts, I=4096 intermediate,
top_k=2, fp32 reference.

Strategy (tensor-parallel over intermediate_size, sharded inside kernel()):
  - Host: softmax + top-k routing; gather all routed token columns into one
    expert-sorted [H, 8192] bf16 matrix (same for every core); slice w1/w2
    along the intermediate dim so core r owns columns [r*I/8, (r+1)*I/8) of
    every expert.  Zero padding: matmul token chunks use exact run lengths.
  - Precision: MM1 and the activation always run in bf16.  Per expert, the
    512 pairs with the smallest combine weights (~50% of pairs, but a small
    share of the output norm) run MM2 in fp8(e4m3) with DoubleRow matmuls
    (2 contraction tiles per PE pass, ~1.7x); the chunk size of exactly 512
    keeps DoubleRow out of its LDWEIGHTS-bound regime.  The w2 fp8 scale
    (1024) is undone on the host via the combine weights.
  - Device (identical program on all 8 cores; only DRAM contents differ):
    per chunk: MM1 with tokens as the moving dim, SiLU*up on scalar/vector
    engines, MM2 (bf16 or fp8-DR) producing the partial y in [h, token]
    layout, downcast to bf16 and DMA out.  Chunks are software-pipelined so
    the tensor engine never waits on the activation step; weight slabs are
    pre-tiled on the host so each is one cheap contiguous DMA (descriptor
    generation on the sync/scalar queues is the scarce resource).
  - Host: sum the 8 partial y matrices, scale columns by combine weights,
    scatter-add into the full [T, H] fp32 output.
"""

import time

import numpy as np
import ml_dtypes

from concourse import bass, bacc, tile, mybir
from concourse.bass_utils import run_bass_kernel_spmd

# Problem dims (hardcoded per contract)
T, H, E, I = 4096, 2048, 8, 4096
P = 128
KH = H // P          # 16 contraction tiles for MM1
IS = I // 8          # 512: per-core intermediate slice
NJJ = IS // P        # 4 act j-tiles per core
KJ = IS // P         # 4 contraction tiles for MM2
NHC = H // P         # 16 output h-tiles
CHUNK = 512

DEM = 512            # smallest-gate pairs per expert whose MM2 runs in fp8
SW2 = 1024.0         # w2 fp8 quant scale (acts are quantized at scale 1)
F8MAX = 240.0        # TRN FP8_EXP4 max normal

_BF16 = mybir.dt.bfloat16
_F32 = mybir.dt.float32
_F8 = mybir.dt.float8e4
_F8_NP = ml_dtypes.float8_e4m3


def split_count(c):
    """Balanced chunk sizes <= CHUNK covering exactly c."""
    if c <= 0:
        return []
    nch = -(-c // CHUNK)
    base, extra = divmod(c, nch)
    return [base + (1 if i < extra else 0) for i in range(nch)]


def make_plan(keeps, dems):
    """Column order per expert: [keep cols][demoted cols].
    Chunks: (expert, off, n, new_expert, is_f8)."""
    chunks = []
    off = 0
    for e in range(E):
        first = True
        for n in split_count(int(keeps[e])):
            chunks.append((e, off, n, first, False))
            off += n
            first = False
        for n in split_count(int(dems[e])):
            chunks.append((e, off, n, first, True))
            off += n
            first = False
    return chunks


def build_kernel(chunks, tt):
    nc = bacc.Bacc("TRN2", target_bir_lowering=False, debug=False, num_devices=8)
    xg_d = nc.dram_tensor("xg", [H, tt], _BF16, kind="ExternalInput").ap()
    # pre-tiled on host: row e*128+p, col k*(2*IS)+j  (j: gate IS then up IS)
    w1t_d = nc.dram_tensor("w1t", [E * P, KH * 2 * IS], _BF16,
                           kind="ExternalInput").ap()
    # pre-tiled on host: row e*128+p, col kj*H+h
    w2t_d = nc.dram_tensor("w2t", [E * P, KJ * H], _BF16,
                           kind="ExternalInput").ap()
    w2f8_d = nc.dram_tensor("w2f8", [E * P, KJ, H], _F8,
                            kind="ExternalInput").ap()
    yg_d = nc.dram_tensor("yg", [H, tt], _BF16, kind="ExternalOutput").ap()

    AF = mybir.ActivationFunctionType

    with tile.TileContext(nc) as tc:
        with (
            tc.tile_pool(name="xp", bufs=3) as xp,
            tc.tile_pool(name="w1p", bufs=2) as w1p,
            tc.tile_pool(name="w2p", bufs=2) as w2p,
            tc.tile_pool(name="w28p", bufs=2) as w28p,
            tc.tile_pool(name="actp", bufs=2) as actp,
            tc.tile_pool(name="act8p", bufs=2) as act8p,
            tc.tile_pool(name="sp", bufs=3) as sp,
            tc.tile_pool(name="op", bufs=3) as op,
            tc.tile_pool(name="psA", bufs=2, space="PSUM") as psA,
            tc.tile_pool(name="psB", bufs=2, space="PSUM") as psB,
        ):
            def load_w1(e, fine=False):
                w1s = w1p.tile([P, KH * 2 * IS], _BF16, tag="w1", name=f"w1_{e}")
                nq = 4 if fine else 1
                step = KH * 2 * IS // nq
                for q in range(nq):
                    nc.scalar.dma_start(
                        out=w1s[:, q * step:(q + 1) * step],
                        in_=w1t_d[e * P:(e + 1) * P, q * step:(q + 1) * step])
                return w1s

            def load_w2(e):
                w2s = w2p.tile([P, KJ * H], _BF16, tag="w2", name=f"w2_{e}")
                nc.scalar.dma_start(out=w2s[:],
                                    in_=w2t_d[e * P:(e + 1) * P, :])
                w28s = w28p.tile([P, KJ, H], _F8, tag="w28", name=f"w28_{e}")
                nc.scalar.dma_start(out=w28s[:],
                                    in_=w2f8_d[e * P:(e + 1) * P])
                return w2s, w28s

            def load_x(ci, fine=False):
                _, off, n, _, _ = chunks[ci]
                xt = xp.tile([P, KH, CHUNK], _BF16, tag="x", name=f"x_{ci}")
                nq = 4
                for q in range(nq):
                    k0, k1 = q * 4, (q + 1) * 4
                    nc.sync.dma_start(
                        out=xt[:, k0:k1, :n],
                        in_=xg_d[k0 * P:k1 * P, off:off + n].rearrange(
                            "(k p) n -> p k n", p=P))
                return xt

            nchunks = len(chunks)
            # expert order as they appear in chunks -> next expert to prefetch
            eorder = []
            for c in chunks:
                if c[3]:
                    eorder.append(c[0])
            next_expert = {a: b for a, b in zip(eorder, eorder[1:])}
            slab_cur = (load_w1(chunks[0][0], fine=True),
                        *load_w2(chunks[0][0]))
            slab_next = None
            xtiles = {0: load_x(0, fine=True)}
            if nchunks > 1:
                xtiles[1] = load_x(1)

            pending = None   # (act tile, w2 slab, off, n, is_f8) awaiting MM2
            for ci, (e, off, n, first, is_f8) in enumerate(chunks):
                if first and ci > 0:
                    slab_cur = slab_next
                # prefetch the next expert's weights a full expert early:
                # issued at this expert's first chunk, needed ~2 chunks later
                if first and e in next_expert:
                    en = next_expert[e]
                    slab_next = (load_w1(en), *load_w2(en))
                w1s, w2s, w28s = slab_cur
                xt = xtiles.pop(ci)
                if ci + 2 < nchunks:
                    xtiles[ci + 2] = load_x(ci + 2)

                adt = _F8 if is_f8 else _BF16
                apool = act8p if is_f8 else actp
                at = apool.tile([P, NJJ, CHUNK], adt,
                                tag="act8" if is_f8 else "act",
                                name=f"act_{ci}")
                for jj in range(NJJ):
                    pg = psA.tile([P, CHUNK], _F32, tag="pg",
                                  name=f"pg_{ci}_{jj}")
                    pu = psA.tile([P, CHUNK], _F32, tag="pu",
                                  name=f"pu_{ci}_{jj}")
                    for k in range(KH):
                        g0 = k * 2 * IS + jj * P
                        nc.tensor.matmul(pg[:, :n], w1s[:, g0:g0 + P],
                                         xt[:, k, :n],
                                         start=(k == 0), stop=(k == KH - 1))
                    for k in range(KH):
                        u0 = k * 2 * IS + IS + jj * P
                        nc.tensor.matmul(pu[:, :n], w1s[:, u0:u0 + P],
                                         xt[:, k, :n],
                                         start=(k == 0), stop=(k == KH - 1))
                    st = sp.tile([P, CHUNK], _F32, tag="silu")
                    nc.scalar.activation(st[:, :n], pg[:, :n], AF.Silu)
                    if is_f8:
                        # fp32->e4m3 conversion via the scalar engine
                        tm = sp.tile([P, CHUNK], _F32, tag="tm")
                        nc.vector.tensor_mul(tm[:, :n], st[:, :n], pu[:, :n])
                        nc.scalar.copy(at[:, jj, :n], tm[:, :n])
                    else:
                        nc.vector.tensor_mul(at[:, jj, :n], st[:, :n],
                                             pu[:, :n])

                # MM2 for the previous chunk (PE stays busy on MM1 above
                # while this chunk's act is produced)
                if pending is not None:
                    emit_mm2(nc, psB, op, yg_d, *pending)
                pending = (at, w28s if is_f8 else w2s, off, n, is_f8)

            emit_mm2(nc, psB, op, yg_d, *pending)
    nc.compile()
    return nc


def emit_mm2(nc, psB, op, yg_d, at, w2s, off, n, is_f8):
    DRm = mybir.MatmulPerfMode.DoubleRow
    HQ = NHC // 4
    ot = None
    for hc in range(NHC):
        if hc % HQ == 0:
            # quarter-sized out tiles: each DMAs out as soon as it fills,
            # keeping SBUF small and overlapping the output transfer
            ot = op.tile([P, HQ, CHUNK], _BF16, tag="out",
                         name=f"out_{off}_{hc // HQ}")
        po = psB.tile([P, CHUNK], _F32, tag="po", name=f"po_{off}_{hc}")
        if is_f8:
            for kp in range(KJ // 2):
                ksl = slice(2 * kp, 2 * kp + 2)
                nc.tensor.matmul(po[:, :n], w2s[:, ksl, hc * P:(hc + 1) * P],
                                 at[:, ksl, :n], start=(kp == 0),
                                 stop=(kp == KJ // 2 - 1), perf_mode=DRm)
        else:
            for kj in range(KJ):
                c0 = kj * H + hc * P
                nc.tensor.matmul(po[:, :n], w2s[:, c0:c0 + P], at[:, kj, :n],
                                 start=(kj == 0), stop=(kj == KJ - 1))
        # alternate engines for the PSUM->SBUF downcast copy
        if hc % 2 == 0:
            nc.scalar.copy(ot[:, hc % HQ, :n], po[:, :n])
        else:
            nc.vector.tensor_copy(ot[:, hc % HQ, :n], po[:, :n])
        if hc % HQ == HQ - 1:
            h0 = hc - HQ + 1
            nc.scalar.dma_start(
                out=yg_d[h0 * P:(hc + 1) * P, off:off + n].rearrange(
                    "(hc p) n -> p hc n", p=P),
                in_=ot[:, :, :n])


_NC_CACHE = {}
LAST_RESULTS = []   # BassKernelResults of each wave of the last kernel() call


def _get_nc(chunks, tt):
    key = (tuple(chunks), tt)
    if key not in _NC_CACHE:
        _NC_CACHE[key] = build_kernel(chunks, tt)
    return _NC_CACHE[key]


def _route(router_logits, top_k):
    """Host routing: stable softmax + top-k (ties broken by lower index,
    matching jax.lax.top_k)."""
    logits = np.asarray(router_logits, dtype=np.float32)
    m = logits.max(axis=-1, keepdims=True)
    p = np.exp(logits - m)
    p /= p.sum(axis=-1, keepdims=True)
    ids = np.argsort(-p, axis=-1, kind="stable")[:, :top_k]   # [T, k]
    gates = np.take_along_axis(p, ids, axis=-1)               # [T, k]
    return ids, gates


def _quant_f8(a, scale):
    return np.clip(a * scale, -F8MAX, F8MAX).astype(_F8_NP)


def _tile_rows(mat, kt):
    """[K*128, N] -> [128, kt*N] with col = k*N + n (slab layout)."""
    kk, nn = mat.shape
    assert kk == kt * P
    return mat.reshape(kt, P, nn).transpose(1, 0, 2).reshape(P, kt * nn)


def kernel(hidden_states, router_logits, w1, w2, top_k):
    top_k = int(top_k)
    x = np.asarray(hidden_states, dtype=np.float32)
    w1 = np.asarray(w1, dtype=np.float32)
    w2 = np.asarray(w2, dtype=np.float32)
    n_tok, hidden = x.shape
    n_exp = w1.shape[0]
    assert (n_tok, hidden, n_exp) == (T, H, E), "compiled for fixed shapes"

    ids, gates = _route(router_logits, top_k)

    # flatten pairs; per expert, demote the DEM smallest-gate pairs
    expert_of = ids.ravel()
    token_of = np.repeat(np.arange(n_tok, dtype=np.int64), top_k)
    gate_of = gates.ravel().astype(np.float32)
    counts = np.bincount(expert_of, minlength=n_exp)
    dem_of = np.zeros(len(gate_of), dtype=np.int64)
    for e in range(n_exp):
        idx = np.where(expert_of == e)[0]
        nd = min(DEM, len(idx))
        small = idx[np.argsort(gate_of[idx], kind="stable")[:nd]]
        dem_of[small] = 1
    order = np.argsort(expert_of * 2 + dem_of, kind="stable")
    token_sorted = token_of[order]
    gate_sorted = gate_of[order]
    dem_sorted = dem_of[order].astype(bool)
    dems = np.bincount(expert_of[dem_of == 1], minlength=n_exp)
    keeps = counts - dems
    tt = int(counts.sum())

    chunks = make_plan(keeps, dems)
    nc = _get_nc(chunks, tt)

    # gathered tokens, transposed: [H, tt] bf16 (same array for all cores)
    xg = np.ascontiguousarray(x.T)[:, token_sorted].astype(ml_dtypes.bfloat16)

    # per-core weight slices, pre-tiled into the SBUF slab layout
    in_maps = []
    for r in range(8):
        w1t = np.empty((E * P, KH * 2 * IS), dtype=ml_dtypes.bfloat16)
        w2t = np.empty((E * P, KJ * H), dtype=ml_dtypes.bfloat16)
        w2f8 = np.empty((E * P, KJ, H), dtype=_F8_NP)
        gsl = slice(r * IS, (r + 1) * IS)
        usl = slice(I + r * IS, I + (r + 1) * IS)
        for e in range(E):
            wcols = np.concatenate([w1[e, gsl, :], w1[e, usl, :]], axis=0)
            t1 = _tile_rows(wcols.T, KH)          # [P, KH*2*IS] fp32
            t2 = _tile_rows(w2[e, :, gsl].T, KJ)  # [P, KJ*H] fp32
            w1t[e * P:(e + 1) * P] = t1
            w2t[e * P:(e + 1) * P] = t2
            w2f8[e * P:(e + 1) * P] = _quant_f8(t2, SW2).reshape(P, KJ, H)
        in_maps.append({"xg": xg, "w1t": w1t, "w2t": w2t, "w2f8": w2f8})

    LAST_RESULTS.clear()
    try:
        res = run_bass_kernel_spmd(nc, in_maps, list(range(8)))
    except Exception:
        # transient device wedge has been observed to clear on retry
        time.sleep(2)
        res = run_bass_kernel_spmd(nc, in_maps, list(range(8)))
    LAST_RESULTS.append(res)

    # host reduce: sum partials, apply combine weights, scatter-add
    ysum = res.results[0]["yg"].astype(np.float32)
    for r in range(1, 8):
        ysum += res.results[r]["yg"].astype(np.float32)
    gate_eff = np.where(dem_sorted, gate_sorted / SW2, gate_sorted)
    weighted = (ysum * gate_eff[None, :]).T          # [tt, H]

    out = np.zeros((n_tok, hidden), dtype=np.float32)
    ord2 = np.argsort(token_sorted, kind="stable")
    for k in range(top_k):     # each token appears exactly top_k times
        sel = ord2[k::top_k]
        if k == 0:
            out[token_sorted[sel]] = weighted[sel]
        else:
            out[token_sorted[sel]] += weighted[sel]
    return out
